# revision 3
# baseline (speedup 1.0000x reference)
"""Conv2D 3x3 (stride 1, pad 1) Trainium2 Bass kernel.

Problem: x (32, 64, 64, 64) NCHW fp32, weight (128, 64, 3, 3) OIHW, bias (128,).
Output: (32, 128, 64, 64).

Strategy: data-parallel over batch across 8 cores (4 images/core). The host
pre-pads each image channel into a 66x66 zero-ringed layout (+ tail slack) and
converts x/weights to fp16 (10-bit mantissa; rel-err ~3e-4 vs the 2e-2 budget).
fp16 moving operands stream at 1 cycle/row on the PE (vs 2 for fp32r whose
4-byte elements saturate the SBUF stream port), and fp16 weight loads get the
hardware fast-weight-load path, so per-matmul LDWEIGHTS hides behind the
previous matmul in the PE's reorder window.

On-chip, partitions 0-63 hold the padded channels and partitions 64-127 hold
the same data shifted down one padded row (a second DMA of the same HBM bytes
at offset 66), so a single K=128 matmul contracts two kernel-row taps at once.
Conv = 6 accumulating matmuls per 396-pixel PSUM tile (3 single ky=2 taps with
K=64 + 3 paired ky={0,1} taps with K=128). Bias-add fuses into the PSUM->SBUF
eviction, alternating between the scalar and vector engines; outputs store as
fp16 and are widened to fp32 on the host.
"""

import numpy as np

import concourse.bass as bass
import concourse.mybir as mybir
import concourse.tile as tile
from concourse import bacc
from concourse.bass_utils import run_bass_kernel_spmd

N_CORES = 8
NIMG = 4  # images per core
C = 64  # input channels
H = W = 64
O = 128  # output channels
PW = 66  # padded row length
PH = 66  # padded rows
IMG = PH * PW  # 4356 padded elements per channel per image
QTOT = H * PW  # 4224 output positions in padded indexing (64 rows x 66)
# Row-aligned PSUM tiles: 10 groups of 6 output rows + 1 of 4 rows. Row
# alignment lets the eviction compact away the 2 garbage columns per row so
# the output staging buffer (and its store DMA) is fully contiguous.
TILE_ROWS = [6] * 10 + [4]
NQT = len(TILE_ROWS)  # 11
XCOLS = 4364  # IMG + 8 slack: lower-half matmul reads reach 4358
UPLEN = QTOT + 8  # 4232: columns needed in the shifted upper half
CUT = 34 * PW  # 2244: image-0 first-chunk split so tile 0 starts early
STORE_SPLIT = 36 * W  # store rows 0-35 (tiles 0-5) while 6-10 compute

F16 = mybir.dt.float16
F32 = mybir.dt.float32

_CACHED_NC = None


def build_nc():
    nc = bacc.Bacc()
    x_in = nc.declare_dram_parameter("xp", [NIMG, C, XCOLS], F16, isOutput=False)
    w_in = nc.declare_dram_parameter("wcat", [2 * C, 6, O], F16, isOutput=False)
    b_in = nc.declare_dram_parameter("bias", [O, 1], F32, isOutput=False)
    out = nc.declare_dram_parameter("out", [NIMG, O, H, W], F16, isOutput=True)

    with tile.TileContext(nc) as tc:
        with (
            tc.tile_pool(name="const", bufs=1) as const_pool,
            tc.tile_pool(name="xp", bufs=4) as x_pool,
            tc.tile_pool(name="osb", bufs=2) as o_pool,
            tc.tile_pool(name="psum", bufs=8, space="PSUM") as psum_pool,
        ):
            wcat = const_pool.tile([2 * C, 6, O], F16)
            bias_t = const_pool.tile([O, 1], F32)
            nc.scalar.dma_start(wcat[:, :, :], w_in[:, :, :])
            nc.scalar.dma_start(bias_t[:, :], b_in[:, :])

            for m in range(NIMG):
                xt = x_pool.tile([128, XCOLS], F16)
                # lower half: padded image; upper half: same shifted one
                # padded row (pairs kernel rows ky=0/1 in one K=128 matmul).
                # Separate queues (SP HWDGE vs GPSIMD SWDGE) so the two loads
                # run concurrently and never queue behind output stores.
                if m == 0:
                    # split image 0 so tile 0's matmuls only wait on the
                    # first ~half of the image (tiles 0-4 read cols < CUT)
                    nc.sync.dma_start(xt[0:C, 0:CUT], x_in[m, :, 0:CUT])
                    nc.gpsimd.dma_start(
                        xt[C : 2 * C, 0:CUT], x_in[m, :, PW : PW + CUT]
                    )
                    nc.sync.dma_start(xt[0:C, CUT:XCOLS], x_in[m, :, CUT:XCOLS])
                    nc.gpsimd.dma_start(
                        xt[C : 2 * C, CUT:UPLEN],
                        x_in[m, :, PW + CUT : PW + UPLEN],
                    )
                else:
                    nc.sync.dma_start(xt[0:C, :], x_in[m, :, :])
                    nc.gpsimd.dma_start(
                        xt[C : 2 * C, 0:UPLEN], x_in[m, :, PW : PW + UPLEN]
                    )

                osb = o_pool.tile([O, H * W], F16)
                r0 = 0
                for t in range(NQT):
                    rows = TILE_ROWS[t]
                    q0 = r0 * PW
                    qt = rows * PW
                    # full 512-fp32 bank: a psum tile that straddles a bank
                    # boundary halves the PE stream rate (2 cycles/row)
                    acc = psum_pool.tile([O, 512], F32, tag="acc")
                    # ky=2 singles first: they read only the lower xt half,
                    # so image-0 startup doesn't wait on the upper-half DMA.
                    for kx in range(3):
                        nc.tensor.matmul(
                            acc[:, 0:qt],
                            wcat[0:C, 3 + kx, :],
                            xt[0:C, q0 + 2 * PW + kx : q0 + 2 * PW + kx + qt],
                            start=(kx == 0),
                            stop=False,
                        )
                    for kx in range(3):
                        nc.tensor.matmul(
                            acc[:, 0:qt],
                            wcat[:, kx, :],
                            xt[0 : 2 * C, q0 + kx : q0 + kx + qt],
                            start=False,
                            stop=(kx == 2),
                        )
                    # evict + bias add, dropping the 2 garbage columns per
                    # row so osb is contiguous valid data. Alternate scalar/
                    # vector engines so eviction never gates PSUM recycling.
                    av = acc[:, 0:qt].rearrange("p (r c) -> p r c", c=PW)
                    ov = osb[:, r0 * W : (r0 + rows) * W].rearrange(
                        "p (r c) -> p r c", c=W
                    )
                    if t % 2 == 0:
                        nc.scalar.activation(
                            ov[:, :, :],
                            av[:, :, 0:W],
                            mybir.ActivationFunctionType.Identity,
                            bias=bias_t[:, :],
                        )
                    else:
                        nc.vector.tensor_scalar_add(
                            ov[:, :, :], av[:, :, 0:W], bias_t[:, 0:1]
                        )
                    r0 += rows
                    if r0 * W == STORE_SPLIT:
                        nc.scalar.dma_start(
                            out[m, :, 0 : STORE_SPLIT // W, :],
                            osb[:, 0:STORE_SPLIT].rearrange(
                                "p (r c) -> p r c", c=W
                            ),
                        )

                nc.scalar.dma_start(
                    out[m, :, STORE_SPLIT // W : H, :],
                    osb[:, STORE_SPLIT : H * W].rearrange("p (r c) -> p r c", c=W),
                )

    nc.compile()
    return nc


def _prep_inputs(x, weight, bias):
    x = np.asarray(x, dtype=np.float32)
    n = x.shape[0]
    z = np.zeros((n, C, PH, PW), dtype=np.float16)
    z[:, :, 1 : 1 + H, 1 : 1 + W] = x
    xp = np.zeros((n, C, XCOLS), dtype=np.float16)
    xp[:, :, :IMG] = z.reshape(n, C, IMG)

    w_t = np.asarray(weight, dtype=np.float32).astype(np.float16)
    w_t = w_t.transpose(1, 2, 3, 0)  # [C, ky, kx, O]
    wcat = np.zeros((2 * C, 6, O), dtype=np.float16)
    wcat[0:C, 0:3, :] = w_t[:, 0, :, :]  # ky=0 (lower half of pairs)
    wcat[C : 2 * C, 0:3, :] = w_t[:, 1, :, :]  # ky=1 (upper half of pairs)
    wcat[0:C, 3:6, :] = w_t[:, 2, :, :]  # ky=2 singles
    b = np.ascontiguousarray(np.asarray(bias, dtype=np.float32).reshape(O, 1))
    return xp, wcat, b


def _in_maps(x, weight, bias):
    xp, wcat, b = _prep_inputs(x, weight, bias)
    return [
        {"xp": xp[i * NIMG : (i + 1) * NIMG], "wcat": wcat, "bias": b}
        for i in range(N_CORES)
    ]


def kernel(x: np.ndarray, weight: np.ndarray, bias: np.ndarray) -> np.ndarray:
    global _CACHED_NC
    if _CACHED_NC is None:
        _CACHED_NC = build_nc()
    res = run_bass_kernel_spmd(_CACHED_NC, _in_maps(x, weight, bias), list(range(N_CORES)))
    return np.concatenate(
        [r["out"].astype(np.float32) for r in res.results], axis=0
    )


def run_profiled(x, weight, bias, tmpdir=None):
    """Dev helper: run with NTFF tracing, return BassKernelResults."""
    global _CACHED_NC
    if _CACHED_NC is None:
        _CACHED_NC = build_nc()
    return run_bass_kernel_spmd(
        _CACHED_NC, _in_maps(x, weight, bias), list(range(N_CORES)),
        trace=True, tmpdir=tmpdir,
    )


# revision 4
# speedup vs baseline: 1.4513x; 1.4513x over previous
"""Conv2D 3x3 (stride 1, pad 1) Trainium2 Bass kernel.

Problem: x (32, 64, 64, 64) NCHW fp32, weight (128, 64, 3, 3) OIHW, bias (128,).
Output: (32, 128, 64, 64).

Strategy: data-parallel over batch across 8 cores (4 images/core). The host
pre-pads each image channel into a 66x66 zero-ringed layout (+ tail slack) and
converts x/weights to fp16 (10-bit mantissa; rel-err ~3e-4 vs the 2e-2 budget).
fp16 moving operands stream at 1 cycle/row on the PE (vs 2 for fp32r whose
4-byte elements saturate the SBUF stream port), and fp16 weight loads get the
hardware fast-weight-load path, so per-matmul LDWEIGHTS hides behind the
previous matmul in the PE's reorder window.

On-chip, partitions 0-63 hold the padded channels and partitions 64-127 hold
the same data shifted down one padded row (a second DMA of the same HBM bytes
at offset 66), so a single K=128 matmul contracts two kernel-row taps at once.
Conv = 6 accumulating matmuls per 396-pixel PSUM tile (3 single ky=2 taps with
K=64 + 3 paired ky={0,1} taps with K=128). Bias-add fuses into the PSUM->SBUF
eviction, alternating between the scalar and vector engines; outputs store as
fp16 and are widened to fp32 on the host.
"""

import numpy as np

import concourse.bass as bass
import concourse.mybir as mybir
import concourse.tile as tile
from concourse import bacc
from concourse.bass_utils import run_bass_kernel_spmd

N_CORES = 8
NIMG = 4  # images per core
C = 64  # input channels
H = W = 64
O = 128  # output channels
PW = 66  # padded row length
PH = 66  # padded rows
IMG = PH * PW  # 4356 padded elements per channel per image
QTOT = H * PW  # 4224 output positions in padded indexing (64 rows x 66)
# Row-aligned PSUM tiles: 10 groups of 6 output rows + 1 of 4 rows. Row
# alignment lets the eviction compact away the 2 garbage columns per row so
# the output staging buffer (and its store DMA) is fully contiguous.
TILE_ROWS = [6] * 10 + [4]
NQT = len(TILE_ROWS)  # 11
XCOLS = 4364  # IMG + 8 slack: lower-half matmul reads reach 4358
UPLEN = QTOT + 8  # 4232: columns needed in the shifted upper half
CUT = 34 * PW  # 2244: image-0 first-chunk split so tile 0 starts early
STORE_SPLIT = 36 * W  # store rows 0-35 (tiles 0-5) while 6-10 compute

F16 = mybir.dt.float16
F32 = mybir.dt.float32

_CACHED_NC = None


def build_nc():
    nc = bacc.Bacc()
    x_in = nc.declare_dram_parameter("xp", [NIMG, C, XCOLS], F16, isOutput=False)
    w_in = nc.declare_dram_parameter("wcat", [2 * C, 6, O], F16, isOutput=False)
    b_in = nc.declare_dram_parameter("bias", [O, 1], F32, isOutput=False)
    out = nc.declare_dram_parameter("out", [NIMG, O, H, W], F16, isOutput=True)

    with tile.TileContext(nc) as tc:
        with (
            tc.tile_pool(name="const", bufs=1) as const_pool,
            tc.tile_pool(name="xp", bufs=4) as x_pool,
            tc.tile_pool(name="osb", bufs=2) as o_pool,
            tc.tile_pool(name="psum", bufs=8, space="PSUM") as psum_pool,
        ):
            wcat = const_pool.tile([2 * C, 6, O], F16)
            bias_t = const_pool.tile([O, 1], F32)
            nc.scalar.dma_start(wcat[:, :, :], w_in[:, :, :])
            nc.scalar.dma_start(bias_t[:, :], b_in[:, :])

            # HAM warmup: the conv matmul pattern (accumulation groups with
            # K=64 singles) never registers as "busy" to the PE activity
            # monitor, so from a cold start the whole kernel runs at the
            # throttled 1.2 GHz clock. A burst of independent full-array
            # matmuls on memset scratch trips HAM to K=8/8 (2.4 GHz) during
            # the initial DMA-wait window, costing no critical-path time.
            wdum = const_pool.tile([128, 512], F16)
            nc.vector.memset(wdum[:, :], 0.0)
            warm_accs = [
                psum_pool.tile([O, 512], F32, tag="acc", name=f"warm{i}")
                for i in range(2)
            ]
            for i in range(40):
                nc.tensor.matmul(
                    warm_accs[i % 2][:, :], wdum[:, 0:128], wdum[:, :],
                    start=True, stop=True,
                )

            for m in range(NIMG):
                xt = x_pool.tile([128, XCOLS], F16)
                # lower half: padded image; upper half: same shifted one
                # padded row (pairs kernel rows ky=0/1 in one K=128 matmul).
                # Separate queues (SP HWDGE vs GPSIMD SWDGE) so the two loads
                # run concurrently and never queue behind output stores.
                if m == 0:
                    # split image 0 so tile 0's matmuls only wait on the
                    # first ~half of the image (tiles 0-4 read cols < CUT)
                    nc.sync.dma_start(xt[0:C, 0:CUT], x_in[m, :, 0:CUT])
                    nc.gpsimd.dma_start(
                        xt[C : 2 * C, 0:CUT], x_in[m, :, PW : PW + CUT]
                    )
                    nc.sync.dma_start(xt[0:C, CUT:XCOLS], x_in[m, :, CUT:XCOLS])
                    nc.gpsimd.dma_start(
                        xt[C : 2 * C, CUT:UPLEN],
                        x_in[m, :, PW + CUT : PW + UPLEN],
                    )
                else:
                    nc.sync.dma_start(xt[0:C, :], x_in[m, :, :])
                    nc.gpsimd.dma_start(
                        xt[C : 2 * C, 0:UPLEN], x_in[m, :, PW : PW + UPLEN]
                    )

                osb = o_pool.tile([O, H * W], F16)
                r0 = 0
                for t in range(NQT):
                    rows = TILE_ROWS[t]
                    q0 = r0 * PW
                    qt = rows * PW
                    # full 512-fp32 bank: a psum tile that straddles a bank
                    # boundary halves the PE stream rate (2 cycles/row)
                    acc = psum_pool.tile([O, 512], F32, tag="acc")
                    # ky=2 singles first: they read only the lower xt half,
                    # so image-0 startup doesn't wait on the upper-half DMA.
                    for kx in range(3):
                        nc.tensor.matmul(
                            acc[:, 0:qt],
                            wcat[0:C, 3 + kx, :],
                            xt[0:C, q0 + 2 * PW + kx : q0 + 2 * PW + kx + qt],
                            start=(kx == 0),
                            stop=False,
                        )
                    for kx in range(3):
                        nc.tensor.matmul(
                            acc[:, 0:qt],
                            wcat[:, kx, :],
                            xt[0 : 2 * C, q0 + kx : q0 + kx + qt],
                            start=False,
                            stop=(kx == 2),
                        )
                    # evict + bias add, dropping the 2 garbage columns per
                    # row so osb is contiguous valid data. Alternate scalar/
                    # vector engines so eviction never gates PSUM recycling.
                    av = acc[:, 0:qt].rearrange("p (r c) -> p r c", c=PW)
                    ov = osb[:, r0 * W : (r0 + rows) * W].rearrange(
                        "p (r c) -> p r c", c=W
                    )
                    if t % 2 == 0:
                        nc.scalar.activation(
                            ov[:, :, :],
                            av[:, :, 0:W],
                            mybir.ActivationFunctionType.Identity,
                            bias=bias_t[:, :],
                        )
                    else:
                        nc.vector.tensor_scalar_add(
                            ov[:, :, :], av[:, :, 0:W], bias_t[:, 0:1]
                        )
                    r0 += rows
                    if r0 * W == STORE_SPLIT:
                        nc.scalar.dma_start(
                            out[m, :, 0 : STORE_SPLIT // W, :],
                            osb[:, 0:STORE_SPLIT].rearrange(
                                "p (r c) -> p r c", c=W
                            ),
                        )

                nc.scalar.dma_start(
                    out[m, :, STORE_SPLIT // W : H, :],
                    osb[:, STORE_SPLIT : H * W].rearrange("p (r c) -> p r c", c=W),
                )

    nc.compile()
    return nc


def _prep_inputs(x, weight, bias):
    x = np.asarray(x, dtype=np.float32)
    n = x.shape[0]
    z = np.zeros((n, C, PH, PW), dtype=np.float16)
    z[:, :, 1 : 1 + H, 1 : 1 + W] = x
    xp = np.zeros((n, C, XCOLS), dtype=np.float16)
    xp[:, :, :IMG] = z.reshape(n, C, IMG)

    w_t = np.asarray(weight, dtype=np.float32).astype(np.float16)
    w_t = w_t.transpose(1, 2, 3, 0)  # [C, ky, kx, O]
    wcat = np.zeros((2 * C, 6, O), dtype=np.float16)
    wcat[0:C, 0:3, :] = w_t[:, 0, :, :]  # ky=0 (lower half of pairs)
    wcat[C : 2 * C, 0:3, :] = w_t[:, 1, :, :]  # ky=1 (upper half of pairs)
    wcat[0:C, 3:6, :] = w_t[:, 2, :, :]  # ky=2 singles
    b = np.ascontiguousarray(np.asarray(bias, dtype=np.float32).reshape(O, 1))
    return xp, wcat, b


def _in_maps(x, weight, bias):
    xp, wcat, b = _prep_inputs(x, weight, bias)
    return [
        {"xp": xp[i * NIMG : (i + 1) * NIMG], "wcat": wcat, "bias": b}
        for i in range(N_CORES)
    ]


def kernel(x: np.ndarray, weight: np.ndarray, bias: np.ndarray) -> np.ndarray:
    global _CACHED_NC
    if _CACHED_NC is None:
        _CACHED_NC = build_nc()
    res = run_bass_kernel_spmd(_CACHED_NC, _in_maps(x, weight, bias), list(range(N_CORES)))
    return np.concatenate(
        [r["out"].astype(np.float32) for r in res.results], axis=0
    )


def run_profiled(x, weight, bias, tmpdir=None):
    """Dev helper: run with NTFF tracing, return BassKernelResults."""
    global _CACHED_NC
    if _CACHED_NC is None:
        _CACHED_NC = build_nc()
    return run_bass_kernel_spmd(
        _CACHED_NC, _in_maps(x, weight, bias), list(range(N_CORES)),
        trace=True, tmpdir=tmpdir,
    )


# revision 5
# speedup vs baseline: 1.5827x; 1.0905x over previous
"""Conv2D 3x3 (stride 1, pad 1) Trainium2 Bass kernel.

Problem: x (32, 64, 64, 64) NCHW fp32, weight (128, 64, 3, 3) OIHW, bias (128,).
Output: (32, 128, 64, 64).

Strategy: data-parallel over batch across 8 cores (4 images/core). The host
pre-pads each image channel into a 66x66 zero-ringed layout (+ tail slack) and
converts x/weights to fp16 (10-bit mantissa; rel-err ~3e-4 vs the 2e-2 budget).
fp16 moving operands stream at 1 cycle/row on the PE (vs 2 for fp32r whose
4-byte elements saturate the SBUF stream port), and fp16 weight loads get the
hardware fast-weight-load path, so per-matmul LDWEIGHTS hides behind the
previous matmul in the PE's reorder window.

On-chip, partitions 0-63 hold the padded channels and partitions 64-127 hold
the same data shifted down one padded row (a second DMA of the same HBM bytes
at offset 66), so a single K=128 matmul contracts two kernel-row taps at once.
Conv = 6 accumulating matmuls per 396-pixel PSUM tile (3 single ky=2 taps with
K=64 + 3 paired ky={0,1} taps with K=128). Bias-add fuses into the PSUM->SBUF
eviction, alternating between the scalar and vector engines; outputs store as
fp16 and are widened to fp32 on the host.
"""

import numpy as np

import concourse.bass as bass
import concourse.mybir as mybir
import concourse.tile as tile
from concourse import bacc
from concourse.bass_utils import run_bass_kernel_spmd

N_CORES = 8
NIMG = 4  # images per core
C = 64  # input channels
H = W = 64
O = 128  # output channels
PW = 66  # padded row length
PH = 66  # padded rows
IMG = PH * PW  # 4356 padded elements per channel per image
QTOT = H * PW  # 4224 output positions in padded indexing (64 rows x 66)
# Row-aligned PSUM tiles: 10 groups of 6 output rows + 1 of 4 rows. Row
# alignment lets the eviction compact away the 2 garbage columns per row so
# the output staging buffer (and its store DMA) is fully contiguous.
TILE_ROWS = [6] * 10 + [4]
NQT = len(TILE_ROWS)  # 11
XCOLS = 4364  # IMG + 8 slack: lower-half matmul reads reach 4358
UPLEN = QTOT + 8  # 4232: columns needed in the shifted upper half
CUT = 34 * PW  # 2244: image-0 first-chunk split so tile 0 starts early
STORE_SPLIT = 36 * W  # store rows 0-35 (tiles 0-5) while 6-10 compute

F16 = mybir.dt.float16
F32 = mybir.dt.float32

_CACHED_NC = None


def build_nc():
    nc = bacc.Bacc()
    x_in = nc.declare_dram_parameter("xp", [NIMG, C, XCOLS], F16, isOutput=False)
    w_in = nc.declare_dram_parameter("wcat", [2 * C, 6, O], F16, isOutput=False)
    b_in = nc.declare_dram_parameter("bias", [O, 1], F32, isOutput=False)
    out = nc.declare_dram_parameter("out", [NIMG, O, H, W], F16, isOutput=True)

    with tile.TileContext(nc) as tc:
        with (
            tc.tile_pool(name="const", bufs=1) as const_pool,
            tc.tile_pool(name="xp", bufs=4) as x_pool,
            tc.tile_pool(name="osb", bufs=2) as o_pool,
            tc.tile_pool(name="psum", bufs=8, space="PSUM") as psum_pool,
        ):
            wcat = const_pool.tile([2 * C, 6, O], F16)
            bias_t = const_pool.tile([O, 1], F32)
            nc.scalar.dma_start(wcat[:, :, :], w_in[:, :, :])
            nc.scalar.dma_start(bias_t[:, :], b_in[:, :])

            # HAM warmup: the conv matmul pattern (accumulation groups with
            # K=64 singles) never registers as "busy" to the PE activity
            # monitor, so from a cold start the whole kernel runs at the
            # throttled 1.2 GHz clock. A burst of independent full-array
            # matmuls on memset scratch trips HAM to K=8/8 (2.4 GHz) during
            # the initial DMA-wait window, costing no critical-path time.
            wdum = const_pool.tile([128, 512], F16)
            nc.vector.memset(wdum[:, :], 0.0)
            warm_accs = [
                psum_pool.tile([O, 512], F32, tag="acc", name=f"warm{i}")
                for i in range(2)
            ]
            for i in range(13):
                nc.tensor.matmul(
                    warm_accs[i % 2][:, :], wdum[:, 0:128], wdum[:, :],
                    start=True, stop=True,
                )

            for m in range(NIMG):
                xt = x_pool.tile([128, XCOLS], F16)
                # lower half: padded image; upper half: same shifted one
                # padded row (pairs kernel rows ky=0/1 in one K=128 matmul).
                # Separate queues (SP HWDGE vs GPSIMD SWDGE) so the two loads
                # run concurrently and never queue behind output stores.
                if m == 0:
                    # split image 0 so tile 0's matmuls only wait on the
                    # first ~half of the image (tiles 0-4 read cols < CUT)
                    nc.sync.dma_start(xt[0:C, 0:CUT], x_in[m, :, 0:CUT])
                    nc.gpsimd.dma_start(
                        xt[C : 2 * C, 0:CUT], x_in[m, :, PW : PW + CUT]
                    )
                    nc.sync.dma_start(xt[0:C, CUT:XCOLS], x_in[m, :, CUT:XCOLS])
                    nc.gpsimd.dma_start(
                        xt[C : 2 * C, CUT:UPLEN],
                        x_in[m, :, PW + CUT : PW + UPLEN],
                    )
                else:
                    nc.sync.dma_start(xt[0:C, :], x_in[m, :, :])
                    nc.gpsimd.dma_start(
                        xt[C : 2 * C, 0:UPLEN], x_in[m, :, PW : PW + UPLEN]
                    )

                osb = o_pool.tile([O, H * W], F16)
                r0 = 0
                for t in range(NQT):
                    rows = TILE_ROWS[t]
                    q0 = r0 * PW
                    qt = rows * PW
                    # full 512-fp32 bank: a psum tile that straddles a bank
                    # boundary halves the PE stream rate (2 cycles/row)
                    acc = psum_pool.tile([O, 512], F32, tag="acc")
                    # ky=2 singles first: they read only the lower xt half,
                    # so image-0 startup doesn't wait on the upper-half DMA.
                    for kx in range(3):
                        nc.tensor.matmul(
                            acc[:, 0:qt],
                            wcat[0:C, 3 + kx, :],
                            xt[0:C, q0 + 2 * PW + kx : q0 + 2 * PW + kx + qt],
                            start=(kx == 0),
                            stop=False,
                        )
                    for kx in range(3):
                        nc.tensor.matmul(
                            acc[:, 0:qt],
                            wcat[:, kx, :],
                            xt[0 : 2 * C, q0 + kx : q0 + kx + qt],
                            start=False,
                            stop=(kx == 2),
                        )
                    # evict + bias add, dropping the 2 garbage columns per
                    # row so osb is contiguous valid data. Alternate scalar/
                    # vector engines so eviction never gates PSUM recycling.
                    av = acc[:, 0:qt].rearrange("p (r c) -> p r c", c=PW)
                    ov = osb[:, r0 * W : (r0 + rows) * W].rearrange(
                        "p (r c) -> p r c", c=W
                    )
                    if t % 2 == 0:
                        nc.scalar.activation(
                            ov[:, :, :],
                            av[:, :, 0:W],
                            mybir.ActivationFunctionType.Identity,
                            bias=bias_t[:, :],
                        )
                    else:
                        nc.vector.tensor_scalar_add(
                            ov[:, :, :], av[:, :, 0:W], bias_t[:, 0:1]
                        )
                    r0 += rows
                    if r0 * W == STORE_SPLIT:
                        nc.scalar.dma_start(
                            out[m, :, 0 : STORE_SPLIT // W, :],
                            osb[:, 0:STORE_SPLIT].rearrange(
                                "p (r c) -> p r c", c=W
                            ),
                        )

                nc.scalar.dma_start(
                    out[m, :, STORE_SPLIT // W : H, :],
                    osb[:, STORE_SPLIT : H * W].rearrange("p (r c) -> p r c", c=W),
                )

    nc.compile()
    return nc


def _prep_inputs(x, weight, bias):
    x = np.asarray(x, dtype=np.float32)
    n = x.shape[0]
    z = np.zeros((n, C, PH, PW), dtype=np.float16)
    z[:, :, 1 : 1 + H, 1 : 1 + W] = x
    xp = np.zeros((n, C, XCOLS), dtype=np.float16)
    xp[:, :, :IMG] = z.reshape(n, C, IMG)

    w_t = np.asarray(weight, dtype=np.float32).astype(np.float16)
    w_t = w_t.transpose(1, 2, 3, 0)  # [C, ky, kx, O]
    wcat = np.zeros((2 * C, 6, O), dtype=np.float16)
    wcat[0:C, 0:3, :] = w_t[:, 0, :, :]  # ky=0 (lower half of pairs)
    wcat[C : 2 * C, 0:3, :] = w_t[:, 1, :, :]  # ky=1 (upper half of pairs)
    wcat[0:C, 3:6, :] = w_t[:, 2, :, :]  # ky=2 singles
    b = np.ascontiguousarray(np.asarray(bias, dtype=np.float32).reshape(O, 1))
    return xp, wcat, b


def _in_maps(x, weight, bias):
    xp, wcat, b = _prep_inputs(x, weight, bias)
    return [
        {"xp": xp[i * NIMG : (i + 1) * NIMG], "wcat": wcat, "bias": b}
        for i in range(N_CORES)
    ]


def kernel(x: np.ndarray, weight: np.ndarray, bias: np.ndarray) -> np.ndarray:
    global _CACHED_NC
    if _CACHED_NC is None:
        _CACHED_NC = build_nc()
    res = run_bass_kernel_spmd(_CACHED_NC, _in_maps(x, weight, bias), list(range(N_CORES)))
    return np.concatenate(
        [r["out"].astype(np.float32) for r in res.results], axis=0
    )


def run_profiled(x, weight, bias, tmpdir=None):
    """Dev helper: run with NTFF tracing, return BassKernelResults."""
    global _CACHED_NC
    if _CACHED_NC is None:
        _CACHED_NC = build_nc()
    return run_bass_kernel_spmd(
        _CACHED_NC, _in_maps(x, weight, bias), list(range(N_CORES)),
        trace=True, tmpdir=tmpdir,
    )


# revision 6
# speedup vs baseline: 1.9356x; 1.2229x over previous
"""Conv2D 3x3 (stride 1, pad 1) Trainium2 Bass kernel.

Problem: x (32, 64, 64, 64) NCHW fp32, weight (128, 64, 3, 3) OIHW, bias (128,).
Output: (32, 128, 64, 64).

Strategy: data-parallel over batch across 8 cores (4 images/core). The host
pre-pads each image channel into a 66x66 zero-ringed layout (+ tail slack) and
converts x/weights to fp16 (10-bit mantissa; rel-err ~3e-4 vs the 2e-2 budget).
fp16 moving operands stream at 1 cycle/row on the PE (vs 2 for fp32r whose
4-byte elements saturate the SBUF stream port), and fp16 weight loads get the
hardware fast-weight-load path, so per-matmul LDWEIGHTS hides behind the
previous matmul in the PE's reorder window.

On-chip, partitions 0-63 hold the padded channels and partitions 64-127 hold
the same data shifted down one padded row (a second DMA of the same HBM bytes
at offset 66), so a single K=128 matmul contracts the ky=0/ky=1 taps at once.
The ky=2 taps are K=64 matmuls; output tiles are processed in pairs so tile
A's ky=2 taps run on PE rows 0-63 concurrently with tile B's on rows 64-127
(upper-half duplicate of the ky=2 weights; tile_position row groups), which
the PE executes 2-at-a-time. Net: 2 output tiles per 9 matmul slots instead
of 12.

The PE activity monitor never un-throttles (1.2 -> 2.4 GHz) on this matmul
pattern alone, so a short burst of independent full-array matmuls on memset
scratch warms the clock during the initial DMA-wait window.

Bias-add fuses into the PSUM->SBUF eviction, alternating between the scalar
and vector engines; outputs store as fp16 and are widened to fp32 on the
host.
"""

import numpy as np

import concourse.bass as bass
import concourse.mybir as mybir
import concourse.tile as tile
from concourse import bacc
from concourse.bass_utils import run_bass_kernel_spmd

N_CORES = 8
NIMG = 4  # images per core
C = 64  # input channels
H = W = 64
O = 128  # output channels
PW = 66  # padded row length
PH = 66  # padded rows
IMG = PH * PW  # 4356 padded elements per channel per image
QTOT = H * PW  # 4224 output positions in padded indexing (64 rows x 66)
# Row-aligned PSUM tiles: 10 groups of 6 output rows + 1 of 4 rows, processed
# as 5 pairs + 1 lone tile. Row alignment lets the eviction compact away the
# 2 garbage columns per row so the output staging buffer is contiguous.
TILE_ROWS = [6] * 10 + [4]
NQT = len(TILE_ROWS)  # 11
XCOLS = 4364  # IMG + 8 slack: lower-half matmul reads reach 4358
UPLEN = QTOT + 8  # 4232: columns needed in the shifted upper half
CUT = 34 * PW  # 2244: image-0 first-chunk split so early tiles start early
STORE_SPLIT = 36 * W  # store rows 0-35 (tiles 0-5) while 6-10 compute

F16 = mybir.dt.float16
F32 = mybir.dt.float32

_CACHED_NC = None


def build_nc():
    nc = bacc.Bacc()
    x_in = nc.declare_dram_parameter("xp", [NIMG, C, XCOLS], F16, isOutput=False)
    w_in = nc.declare_dram_parameter("wcat", [2 * C, 9, O], F16, isOutput=False)
    b_in = nc.declare_dram_parameter("bias", [O, 1], F32, isOutput=False)
    out = nc.declare_dram_parameter("out", [NIMG, O, H, W], F16, isOutput=True)

    with tile.TileContext(nc) as tc:
        with (
            tc.tile_pool(name="const", bufs=1) as const_pool,
            tc.tile_pool(name="xp", bufs=4) as x_pool,
            tc.tile_pool(name="osb", bufs=2) as o_pool,
            tc.tile_pool(name="psum", bufs=8, space="PSUM") as psum_pool,
        ):
            wcat = const_pool.tile([2 * C, 9, O], F16)
            bias_t = const_pool.tile([O, 1], F32)
            nc.scalar.dma_start(wcat[:, :, :], w_in[:, :, :])
            nc.scalar.dma_start(bias_t[:, :], b_in[:, :])

            # HAM warmup: the conv matmul pattern (accumulation groups with
            # K=64 singles) never registers as "busy" to the PE activity
            # monitor, so from a cold start the whole kernel runs at the
            # throttled 1.2 GHz clock. A burst of independent full-array
            # matmuls on memset scratch trips HAM to K=8/8 (2.4 GHz) during
            # the initial DMA-wait window, costing no critical-path time.
            wdum = const_pool.tile([128, 512], F16)
            nc.vector.memset(wdum[:, :], 0.0)
            warm_accs = [
                psum_pool.tile([O, 512], F32, tag="acc", name=f"warm{i}")
                for i in range(2)
            ]
            for i in range(13):
                nc.tensor.matmul(
                    warm_accs[i % 2][:, :], wdum[:, 0:128], wdum[:, :],
                    start=True, stop=True,
                )

            def singles(acc, qt, q0, kx, start):
                """ky=2 tap kx for one tile, lower PE rows (partitions 0-63)."""
                nc.tensor.matmul(
                    acc[:, 0:qt],
                    wcat[0:C, 3 + kx, :],
                    xt[0:C, q0 + 2 * PW + kx : q0 + 2 * PW + kx + qt],
                    start=start,
                    stop=False,
                    skip_group_check=True,
                )

            def singles_hi(acc, qt, q0, kx, start):
                """ky=2 tap kx on upper PE rows: partitions 64-127 hold the
                +66-shifted copy, so base col q0+132+kx lives at q0+66+kx."""
                nc.tensor.matmul(
                    acc[:, 0:qt],
                    wcat[C : 2 * C, 6 + kx, :],
                    xt[C : 2 * C, q0 + PW + kx : q0 + PW + kx + qt],
                    start=start,
                    stop=False,
                    skip_group_check=True,
                )

            def pairs(acc, qt, q0):
                """ky=0/ky=1 taps, K=128 across both halves."""
                for kx in range(3):
                    nc.tensor.matmul(
                        acc[:, 0:qt],
                        wcat[:, kx, :],
                        xt[0 : 2 * C, q0 + kx : q0 + kx + qt],
                        start=False,
                        stop=(kx == 2),
                        skip_group_check=True,
                    )

            def evict(acc, qt, rows, r0, t):
                """PSUM->SBUF + bias, dropping the 2 garbage cols per row."""
                av = acc[:, 0:qt].rearrange("p (r c) -> p r c", c=PW)
                ov = osb[:, r0 * W : (r0 + rows) * W].rearrange(
                    "p (r c) -> p r c", c=W
                )
                if t % 2 == 0:
                    nc.scalar.activation(
                        ov[:, :, :],
                        av[:, :, 0:W],
                        mybir.ActivationFunctionType.Identity,
                        bias=bias_t[:, :],
                    )
                else:
                    nc.vector.tensor_scalar_add(
                        ov[:, :, :], av[:, :, 0:W], bias_t[:, 0:1]
                    )

            for m in range(NIMG):
                xt = x_pool.tile([128, XCOLS], F16)
                # lower half: padded image; upper half: same shifted one
                # padded row. Separate queues (SP HWDGE vs GPSIMD SWDGE) so
                # the loads run concurrently and never queue behind stores.
                if m == 0:
                    nc.sync.dma_start(xt[0:C, 0:CUT], x_in[m, :, 0:CUT])
                    nc.gpsimd.dma_start(
                        xt[C : 2 * C, 0:CUT], x_in[m, :, PW : PW + CUT]
                    )
                    nc.sync.dma_start(xt[0:C, CUT:XCOLS], x_in[m, :, CUT:XCOLS])
                    nc.gpsimd.dma_start(
                        xt[C : 2 * C, CUT:UPLEN],
                        x_in[m, :, PW + CUT : PW + UPLEN],
                    )
                else:
                    nc.sync.dma_start(xt[0:C, :], x_in[m, :, :])
                    nc.gpsimd.dma_start(
                        xt[C : 2 * C, 0:UPLEN], x_in[m, :, PW : PW + UPLEN]
                    )

                osb = o_pool.tile([O, H * W], F16)
                # 5 tile pairs: tile A's ky=2 taps (rows 0-63) interleave
                # with tile B's (rows 64-127) and execute concurrently.
                for u in range(5):
                    ta, tb = 2 * u, 2 * u + 1
                    ra, rb = 6 * ta, 6 * tb
                    qa, qb = ra * PW, rb * PW
                    qt = 6 * PW
                    acca = psum_pool.tile([O, 512], F32, tag="acc", name="acca")
                    accb = psum_pool.tile([O, 512], F32, tag="acc", name="accb")
                    for kx in range(3):
                        singles(acca, qt, qa, kx, kx == 0)
                        singles_hi(accb, qt, qb, kx, kx == 0)
                    pairs(acca, qt, qa)
                    pairs(accb, qt, qb)
                    evict(acca, qt, 6, ra, ta)
                    evict(accb, qt, 6, rb, tb)
                    if (rb + 6) * W == STORE_SPLIT:
                        nc.scalar.dma_start(
                            out[m, :, 0 : STORE_SPLIT // W, :],
                            osb[:, 0:STORE_SPLIT].rearrange(
                                "p (r c) -> p r c", c=W
                            ),
                        )
                # lone 4-row tile 10
                rows = TILE_ROWS[10]
                r0 = 60
                q0 = r0 * PW
                qt = rows * PW
                acc = psum_pool.tile([O, 512], F32, tag="acc", name="accl")
                for kx in range(3):
                    singles(acc, qt, q0, kx, kx == 0)
                pairs(acc, qt, q0)
                evict(acc, qt, rows, r0, 10)

                nc.scalar.dma_start(
                    out[m, :, STORE_SPLIT // W : H, :],
                    osb[:, STORE_SPLIT : H * W].rearrange("p (r c) -> p r c", c=W),
                )

    nc.compile()
    return nc


def _prep_inputs(x, weight, bias):
    x = np.asarray(x, dtype=np.float32)
    n = x.shape[0]
    z = np.zeros((n, C, PH, PW), dtype=np.float16)
    z[:, :, 1 : 1 + H, 1 : 1 + W] = x
    xp = np.zeros((n, C, XCOLS), dtype=np.float16)
    xp[:, :, :IMG] = z.reshape(n, C, IMG)

    w_t = np.asarray(weight, dtype=np.float32).astype(np.float16)
    w_t = w_t.transpose(1, 2, 3, 0)  # [C, ky, kx, O]
    wcat = np.zeros((2 * C, 9, O), dtype=np.float16)
    wcat[0:C, 0:3, :] = w_t[:, 0, :, :]  # ky=0 (lower half of pairs)
    wcat[C : 2 * C, 0:3, :] = w_t[:, 1, :, :]  # ky=1 (upper half of pairs)
    wcat[0:C, 3:6, :] = w_t[:, 2, :, :]  # ky=2 singles, lower-row tiles
    wcat[C : 2 * C, 6:9, :] = w_t[:, 2, :, :]  # ky=2 singles, upper-row tiles
    b = np.ascontiguousarray(np.asarray(bias, dtype=np.float32).reshape(O, 1))
    return xp, wcat, b


def _in_maps(x, weight, bias):
    xp, wcat, b = _prep_inputs(x, weight, bias)
    return [
        {"xp": xp[i * NIMG : (i + 1) * NIMG], "wcat": wcat, "bias": b}
        for i in range(N_CORES)
    ]


def kernel(x: np.ndarray, weight: np.ndarray, bias: np.ndarray) -> np.ndarray:
    global _CACHED_NC
    if _CACHED_NC is None:
        _CACHED_NC = build_nc()
    res = run_bass_kernel_spmd(_CACHED_NC, _in_maps(x, weight, bias), list(range(N_CORES)))
    return np.concatenate(
        [r["out"].astype(np.float32) for r in res.results], axis=0
    )


def run_profiled(x, weight, bias, tmpdir=None):
    """Dev helper: run with NTFF tracing, return BassKernelResults."""
    global _CACHED_NC
    if _CACHED_NC is None:
        _CACHED_NC = build_nc()
    return run_bass_kernel_spmd(
        _CACHED_NC, _in_maps(x, weight, bias), list(range(N_CORES)),
        trace=True, tmpdir=tmpdir,
    )
